# revision 12
# baseline (speedup 1.0000x reference)
"""Trainium2 Bass kernel for nn_Bilinear_54065048322517.

Math:  out[b, j] = input2[b, j] * sum_{i,k} weights[i, j, k] * input1[b, i]
           =   input2 * (input1 @ weights.sum(axis=2))
Shapes: input1 (16384, 64) f32, input2 (16384, 2048) f32,
        weights (64, 2048, 64) f32, out (16384, 2048) f32.

Sharding: split J=2048 into 8 shards of 256 (one per NeuronCore);
J-sharding avoids replicating the 32MB weights tensor.

Precision: the HBM side runs in bf16, halving traffic to 20MB per
core: input1 2MB + input2 shard 8MB + weights shard 2MB read, out
shard 8MB written.  The K-reduction and GEMM accumulate in f32 (DVE
reduce -> f32, matmul -> PSUM f32); bf16 rounding enters only on the
stored operands and the final output (rel-l2 vs the f32 reference
~4e-3, well under the 2e-2 gate).  The host casts/gathers.

Host staging puts every per-core DRAM array in the exact layout the
kernel consumes, so all DMAs move 2-16KB contiguous runs/partition:
  x1t[q*64+i, n*128+p]       = input1[n*256 + 2p + q, i]
  wd[h*64+i, j''*64+k]       = weights[i, jsl(h*128+j''), k]
  x2d[g, p, (s*2+q)*256+j]   = input2[(g*8+s)*256 + 2p + q, jsl(j)]
  (out is produced in the x2d layout and un-permuted on the host)

Per-core kernel:
  - wd loaded in 8 chunks of 256KB alternating between the two HWDGE
    rings (scalar/ACT gets even chunks, sync/SP odd), each reduced
    over k by DVE as it lands -> w2h (128, 128) f32, partition h*64+i
    holds w2[i, h-half cols].  Pipelining the reduce under the loads
    keeps the w2 chain off the critical path (v1 lost ~12us here).
  - two selection-matrix matmuls (sel_h[h*64+i, q*64+i] = 1) fan
    w2h out to pdup[q*64+i, h*128+j''] = w2[i, j'] (PSUM f32); one
    DVE copy casts to w2dup (128, 256) bf16 so each PE row-group q
    holds the full reduced weight matrix.
  - x1t loads on the sync ring behind the odd w chunks (0.5MB head
    chunk covering groups 0-1 first, then the 1.5MB rest).
  - 8 groups of 8 super-tiles (256 rows each): per group one 1MB x2
    load (scalar ring); per super-tile PAIR four matmuls (q=0/q=1 on
    disjoint PE row-groups run concurrently; the two u-steps share a
    PSUM bank half each: col q*512 + u*256) into a 2-bank PSUM tile,
    then ONE DVE multiply for the pair (1024 elems/partition -- half
    the per-instruction read-write-bubble overhead of per-super-tile
    muls); two 512KB half-group stores (sync ring).
"""

import numpy as np

B, I, J, K = 16384, 64, 2048, 64
NCORES = 8
JS = J // NCORES          # 256 columns per core
NSUP = B // 256           # 64 super-tiles of 256 rows
GROUP = 8                 # super-tiles per DMA group
NG = NSUP // GROUP        # 8 groups
NPAIR = GROUP // 2        # super-tile pairs per group
XBUFS = 4                 # xtile buffer depth
OBUFS = 3                 # otile buffer depth
NWCHUNK = 4               # weights load chunks (one per j'-quarter)
GFREE = GROUP * 2 * JS    # 4096 elems per partition per group

_CACHE = {}


def _build_nc():
    from contextlib import ExitStack

    import concourse.mybir as mybir
    import concourse.tile as tile
    from concourse import bacc

    f32 = mybir.dt.float32
    bf16 = mybir.dt.bfloat16
    nc = bacc.Bacc()

    x1 = nc.dram_tensor("input1", [128, NSUP * 128], bf16, kind="ExternalInput")
    x2 = nc.dram_tensor("input2", [NG, 128, GFREE], bf16, kind="ExternalInput")
    w = nc.dram_tensor(
        "weights", [NWCHUNK, 128, JS * K // NWCHUNK // 2], bf16, kind="ExternalInput"
    )
    out = nc.dram_tensor("out", [NG, 128, GFREE], bf16, kind="ExternalOutput")

    with tile.TileContext(nc) as tc, ExitStack() as ctx:
        const_pool = ctx.enter_context(tc.tile_pool(name="const", bufs=1))
        wc_pool = ctx.enter_context(tc.tile_pool(name="wc", bufs=1))
        x_pool = ctx.enter_context(tc.tile_pool(name="xin", bufs=XBUFS))
        o_pool = ctx.enter_context(tc.tile_pool(name="oout", bufs=OBUFS))
        yb_pool = ctx.enter_context(tc.tile_pool(name="yb", bufs=3))
        ps_pool = ctx.enter_context(tc.tile_pool(name="ps", bufs=3, space="PSUM"))
        tr_pool = ctx.enter_context(tc.tile_pool(name="tr", bufs=1, space="PSUM"))

        # selection masks: sel_h[h*64+i, q*64+i] = 1, else 0
        # (sel_h.T @ w2h)[q*64+i, j''] = w2h[h*64+i, j'']
        sel = []
        for h in range(2):
            sh = const_pool.tile([128, 128], f32, name=f"sel{h}")
            sel.append(sh)
            nc.gpsimd.memset(sh[:], 0.0)
            for q in range(2):
                # fill where p - 64h - m' == 0 over the (128, 64) column block
                nc.gpsimd.affine_select(
                    out=sh[:, q * 64 : (q + 1) * 64],
                    in_=sh[:, q * 64 : (q + 1) * 64],
                    compare_op=mybir.AluOpType.not_equal,
                    fill=1.0,
                    base=-64 * h,
                    pattern=[[-1, 64]],
                    channel_multiplier=1,
                )

        # ---- weights: 4 chunk-major 512KB loads (4KB descriptors, two
        # per ring so w gets both rings' full bandwidth first), each
        # reduced over k by DVE as it lands.  Chunk c holds j'-quarter
        # [c*64, (c+1)*64): partition s*64+i (s = sub-half), free t*64+k
        # with j' = c*64 + s*32 + t. ----
        wcsz = JS * K // NWCHUNK // 2  # 2048 elems per partition per chunk
        tq = JS // NWCHUNK // 2        # 32 j'-values per (chunk, s)
        w2cs = []
        wchunks = []
        for c in range(NWCHUNK):
            wchunk = wc_pool.tile([128, wcsz], bf16, name=f"wchunk{c}", tag=f"wc{c}")
            wchunks.append(wchunk)
            eng = nc.scalar if c % 2 == 0 else nc.sync
            eng.dma_start(out=wchunk[:], in_=w[c])
        for c in range(NWCHUNK):
            w2c = const_pool.tile([128, tq], f32, name=f"w2c{c}")
            w2cs.append(w2c)
            nc.vector.tensor_reduce(
                out=w2c[:],
                in_=wchunks[c][:].rearrange("p (t k) -> p t k", k=K),
                axis=mybir.AxisListType.X,
                op=mybir.AluOpType.add,
            )

        # ---- x1t load on sync ring (head chunk first for groups 0-1) ----
        x1T = const_pool.tile([128, NSUP * 128], bf16)
        nc.sync.dma_start(out=x1T[:, 0:2048], in_=x1[:, 0:2048])
        nc.sync.dma_start(out=x1T[:, 2048:], in_=x1[:, 2048:])

        # ---- x2 group prefetch (scalar ring) ----
        xtiles = []

        def load(g):
            assert len(xtiles) == g
            xt = x_pool.tile([128, GFREE], bf16, name=f"xt{g}", tag="xt")
            xtiles.append(xt)
            nc.scalar.dma_start(out=xt[:], in_=x2[g])

        for g in range(XBUFS):
            load(g)

        # ---- de-interleave + duplicate w2 via selection matmuls ----
        # pdup[q*64+i, c*64+s*32+t] = w2c_c[s*64+i, t]
        pdup = tr_pool.tile([128, JS], f32)
        for c in range(NWCHUNK):
            for s in range(2):
                nc.tensor.matmul(
                    pdup[:, c * 64 + s * tq : c * 64 + (s + 1) * tq],
                    lhsT=sel[s][:],
                    rhs=w2cs[c][:],
                    start=True,
                    stop=True,
                )
        w2dup = const_pool.tile([128, JS], bf16)
        nc.vector.tensor_copy(out=w2dup[:], in_=pdup[:])

        # ---- main loop ----
        # Per super-tile pair: 4 matmuls -> PSUM f32; ScalarE (ACT, idle
        # otherwise, has the PSUM port) casts the pair to SBUF bf16; DVE
        # multiplies bf16*bf16 -> bf16 at its 2x 16-bit rate.
        def process(g):
            xt = xtiles[g]
            ot = o_pool.tile([128, GFREE], bf16, name=f"ot{g}", tag="ot")
            for mm in range(NPAIR):
                # 4 matmuls into a 2-bank PSUM tile: col q*512 + u*256;
                # the q pair runs concurrently on disjoint PE row-groups
                # into different banks, u-steps fill the bank halves.
                pt = ps_pool.tile([128, 4 * JS], f32)  # (128, 1024)
                for u in range(2):
                    n = (g * NPAIR + mm) * 2 + u
                    for q in range(2):
                        nc.tensor.matmul(
                            pt[:, q * 512 + u * JS : q * 512 + (u + 1) * JS],
                            lhsT=x1T[
                                q * 64 : (q + 1) * 64, n * 128 : (n + 1) * 128
                            ],
                            rhs=w2dup[q * 64 : (q + 1) * 64, :],
                            start=True,
                            stop=True,
                        )
                ot_v = ot[:, mm * 1024 : (mm + 1) * 1024].rearrange(
                    "p (u q j) -> p q u j", u=2, q=2
                )
                xt_v = xt[:, mm * 1024 : (mm + 1) * 1024].rearrange(
                    "p (u q j) -> p q u j", u=2, q=2
                )
                if mm == 0:
                    # direct f32-PSUM multiply on DVE (no ACT copy); one
                    # pair per group keeps DVE and ACT loads balanced
                    nc.vector.tensor_mul(
                        ot_v,
                        pt[:].rearrange("p (q u j) -> p q u j", q=2, u=2),
                        xt_v,
                    )
                else:
                    yb = yb_pool.tile([128, 4 * JS], bf16, name="yb", tag="yb")
                    nc.scalar.copy(yb[:], pt[:])
                    nc.vector.tensor_mul(
                        ot_v,
                        yb[:].rearrange("p (q u j) -> p q u j", q=2, u=2),
                        xt_v,
                    )
                if mm % 2 == 1:
                    half = mm // 2
                    nc.sync.dma_start(
                        out=out[g][:, half * 2048 : (half + 1) * 2048],
                        in_=ot[:, half * 2048 : (half + 1) * 2048],
                    )
            if g + XBUFS < NG:
                load(g + XBUFS)

        for g in range(NG):
            process(g)

    nc.compile()
    return nc


def _get_nc():
    if "nc" not in _CACHE:
        _CACHE["nc"] = _build_nc()
    return _CACHE["nc"]


def _make_in_maps(input1, input2, weights):
    import ml_dtypes

    BF = ml_dtypes.bfloat16
    input1 = np.asarray(input1, dtype=np.float32)
    input2 = np.asarray(input2, dtype=np.float32)
    weights = np.asarray(weights, dtype=np.float32)

    # x1t[q*64+i, n*128+p] = input1[n*256 + 2p + q, i]
    x1t = (
        input1.reshape(NSUP, 128, 2, I)
        .transpose(2, 3, 0, 1)
        .reshape(128, NSUP * 128)
        .astype(BF)
    )

    in_maps = []
    for c in range(NCORES):
        sl = slice(c * JS, (c + 1) * JS)
        # wd[c, s*64+i, t*64+k] = weights[i, jsl(c*64 + s*32 + t), k]
        wd = (
            weights[:, sl, :]
            .reshape(I, NWCHUNK, 2, JS // NWCHUNK // 2, K)
            .transpose(1, 2, 0, 3, 4)
            .reshape(NWCHUNK, 128, JS * K // NWCHUNK // 2)
            .astype(BF)
        )
        # x2d[g, p, (s*2+q)*256+j] = input2[(g*8+s)*256 + 2p + q, sl][j]
        x2d = (
            input2[:, sl]
            .reshape(NG, GROUP, 128, 2, JS)
            .transpose(0, 2, 1, 3, 4)
            .reshape(NG, 128, GFREE)
            .astype(BF)
        )
        in_maps.append({"input1": x1t, "input2": x2d, "weights": wd})
    return in_maps


def run(input1, input2, weights, trace=False, **spmd_kwargs):
    from concourse.bass_utils import run_bass_kernel_spmd

    nc = _get_nc()
    in_maps = _make_in_maps(input1, input2, weights)
    res = run_bass_kernel_spmd(
        nc, in_maps, core_ids=list(range(NCORES)), trace=trace, **spmd_kwargs
    )
    outs = []
    for c in range(NCORES):
        o = np.asarray(res.results[c]["out"])  # (NG, 128, GFREE) bf16
        outs.append(
            o.reshape(NG, 128, GROUP, 2, JS)
            .transpose(0, 2, 1, 3, 4)
            .reshape(B, JS)
        )
    full = np.concatenate(outs, axis=1).astype(np.float32)
    return full, res


def kernel(input1, input2, weights):
    full, _ = run(input1, input2, weights, trace=False)
    return full


# revision 19
# speedup vs baseline: 1.0132x; 1.0132x over previous
"""Trainium2 Bass kernel for nn_Bilinear_54065048322517.

Math:  out[b, j] = input2[b, j] * sum_{i,k} weights[i, j, k] * input1[b, i]
           =   input2 * (input1 @ weights.sum(axis=2))
Shapes: input1 (16384, 64) f32, input2 (16384, 2048) f32,
        weights (64, 2048, 64) f32, out (16384, 2048) f32.

Sharding: split J=2048 into 8 shards of 256 (one per NeuronCore);
J-sharding avoids replicating the 32MB weights tensor.

Precision: the HBM side runs in bf16, halving traffic to 20MB per
core: input1 2MB + input2 shard 8MB + weights shard 2MB read, out
shard 8MB written.  The K-reduction and GEMM accumulate in f32 (DVE
reduce -> f32, matmul -> PSUM f32); bf16 rounding enters only on the
stored operands and the final output (rel-l2 vs the f32 reference
~4e-3, well under the 2e-2 gate).  The host casts/gathers.

Host staging puts every per-core DRAM array in the exact layout the
kernel consumes, so all DMAs move 2-16KB contiguous runs/partition:
  x1t[q*64+i, n*128+p]       = input1[n*256 + 2p + q, i]
  wd[h*64+i, j''*64+k]       = weights[i, jsl(h*128+j''), k]
  x2d[g, p, (s*2+q)*256+j]   = input2[(g*8+s)*256 + 2p + q, jsl(j)]
  (out is produced in the x2d layout and un-permuted on the host)

Per-core kernel:
  - wd loaded in 8 chunks of 256KB alternating between the two HWDGE
    rings (scalar/ACT gets even chunks, sync/SP odd), each reduced
    over k by DVE as it lands -> w2h (128, 128) f32, partition h*64+i
    holds w2[i, h-half cols].  Pipelining the reduce under the loads
    keeps the w2 chain off the critical path (v1 lost ~12us here).
  - two selection-matrix matmuls (sel_h[h*64+i, q*64+i] = 1) fan
    w2h out to pdup[q*64+i, h*128+j''] = w2[i, j'] (PSUM f32); one
    DVE copy casts to w2dup (128, 256) bf16 so each PE row-group q
    holds the full reduced weight matrix.
  - x1t loads on the sync ring behind the odd w chunks (0.5MB head
    chunk covering groups 0-1 first, then the 1.5MB rest).
  - 8 groups of 8 super-tiles (256 rows each): per group one 1MB x2
    load (scalar ring); per super-tile PAIR four matmuls (q=0/q=1 on
    disjoint PE row-groups run concurrently; the two u-steps share a
    PSUM bank half each: col q*512 + u*256) into a 2-bank PSUM tile,
    then ONE DVE multiply for the pair (1024 elems/partition -- half
    the per-instruction read-write-bubble overhead of per-super-tile
    muls); two 512KB half-group stores (sync ring).
"""

import numpy as np

B, I, J, K = 16384, 64, 2048, 64
NCORES = 8
JS = J // NCORES          # 256 columns per core
NSUP = B // 256           # 64 super-tiles of 256 rows
GROUP = 8                 # super-tiles per DMA group
NG = NSUP // GROUP        # 8 groups
NPAIR = GROUP // 2        # super-tile pairs per group
XBUFS = 5                 # xtile buffer depth
OBUFS = 4                 # otile buffer depth
NWCHUNK = 8               # weights load chunks
GFREE = GROUP * 2 * JS    # 4096 elems per partition per group

_CACHE = {}


def _build_nc():
    from contextlib import ExitStack

    import concourse.mybir as mybir
    import concourse.tile as tile
    from concourse import bacc

    f32 = mybir.dt.float32
    bf16 = mybir.dt.bfloat16
    nc = bacc.Bacc()

    x1 = nc.dram_tensor("input1", [128, NSUP * 128], bf16, kind="ExternalInput")
    x2 = nc.dram_tensor("input2", [NG, 128, GFREE], bf16, kind="ExternalInput")
    w = nc.dram_tensor("weights", [128, (JS // 2) * K], bf16, kind="ExternalInput")
    out = nc.dram_tensor("out", [NG, 128, GFREE], bf16, kind="ExternalOutput")

    with tile.TileContext(nc) as tc, ExitStack() as ctx:
        const_pool = ctx.enter_context(tc.tile_pool(name="const", bufs=1))
        wc_pool = ctx.enter_context(tc.tile_pool(name="wc", bufs=1))
        x_pool = ctx.enter_context(tc.tile_pool(name="xin", bufs=XBUFS))
        o_pool = ctx.enter_context(tc.tile_pool(name="oout", bufs=OBUFS))
        yb_pool = ctx.enter_context(tc.tile_pool(name="yb", bufs=3))
        ps_pool = ctx.enter_context(tc.tile_pool(name="ps", bufs=3, space="PSUM"))
        tr_pool = ctx.enter_context(tc.tile_pool(name="tr", bufs=1, space="PSUM"))

        # selection masks: sel_h[h*64+i, q*64+i] = 1, else 0
        # (sel_h.T @ w2h)[q*64+i, j''] = w2h[h*64+i, j'']
        sel = []
        for h in range(2):
            sh = const_pool.tile([128, 128], f32, name=f"sel{h}")
            sel.append(sh)
            nc.gpsimd.memset(sh[:], 0.0)
            for q in range(2):
                # fill where p - 64h - m' == 0 over the (128, 64) column block
                nc.gpsimd.affine_select(
                    out=sh[:, q * 64 : (q + 1) * 64],
                    in_=sh[:, q * 64 : (q + 1) * 64],
                    compare_op=mybir.AluOpType.not_equal,
                    fill=1.0,
                    base=-64 * h,
                    pattern=[[-1, 64]],
                    channel_multiplier=1,
                )

        # ---- weights: 8 chunk loads front-loaded on both rings, then
        # pipelined DVE reduce (chunks 0-3 head the scalar ring, 4-7 the
        # sync ring, so w gets both rings' full bandwidth first) ----
        w2h = const_pool.tile([128, JS // 2], f32)  # (128, 128), part h*64+i
        wcsz = (JS // 2) * K // NWCHUNK  # 1024 elems per partition per chunk
        jcs = (JS // 2) // NWCHUNK       # 16 w2h columns per chunk
        wchunks = []
        for c in range(NWCHUNK):
            wchunk = wc_pool.tile([128, wcsz], bf16, name=f"wchunk{c}", tag=f"wc{c}")
            wchunks.append(wchunk)
            # interleave ring order with landing order: c even -> scalar,
            # c odd -> sync, so reduces can run in emission order c=0..7
            eng = nc.scalar if c % 2 == 0 else nc.sync
            eng.dma_start(out=wchunk[:], in_=w[:, c * wcsz : (c + 1) * wcsz])
        for c in range(NWCHUNK):
            nc.vector.tensor_reduce(
                out=w2h[:, c * jcs : (c + 1) * jcs],
                in_=wchunks[c][:].rearrange("p (j k) -> p j k", k=K),
                axis=mybir.AxisListType.X,
                op=mybir.AluOpType.add,
            )

        # ---- x1t load on sync ring (head chunk first for groups 0-1) ----
        x1T = const_pool.tile([128, NSUP * 128], bf16)
        nc.sync.dma_start(out=x1T[:, 0:2048], in_=x1[:, 0:2048])
        nc.sync.dma_start(out=x1T[:, 2048:], in_=x1[:, 2048:])

        # ---- x2 group prefetch (scalar ring) ----
        xtiles = []

        def load(g):
            assert len(xtiles) == g
            xt = x_pool.tile([128, GFREE], bf16, name=f"xt{g}", tag="xt")
            xtiles.append(xt)
            nc.scalar.dma_start(out=xt[:], in_=x2[g])

        for g in range(XBUFS):
            load(g)

        # ---- de-interleave + duplicate w2 via selection matmuls ----
        # pdup[q*64+i, h*128+j''] = w2h[h*64+i, j'']
        pdup = tr_pool.tile([128, JS], f32)
        for h in range(2):
            nc.tensor.matmul(
                pdup[:, h * 128 : (h + 1) * 128],
                lhsT=sel[h][:],
                rhs=w2h[:],
                start=True,
                stop=True,
            )
        w2dup = const_pool.tile([128, JS], bf16)
        nc.vector.tensor_copy(out=w2dup[:], in_=pdup[:])

        # ---- main loop ----
        # Per super-tile pair: 4 matmuls -> PSUM f32; ScalarE (ACT, idle
        # otherwise, has the PSUM port) casts the pair to SBUF bf16; DVE
        # multiplies bf16*bf16 -> bf16 at its 2x 16-bit rate.
        def process(g):
            xt = xtiles[g]
            ot = o_pool.tile([128, GFREE], bf16, name=f"ot{g}", tag="ot")
            for mm in range(NPAIR):
                # 4 matmuls into a 2-bank PSUM tile: col q*512 + u*256;
                # the q pair runs concurrently on disjoint PE row-groups
                # into different banks, u-steps fill the bank halves.
                pt = ps_pool.tile([128, 4 * JS], f32)  # (128, 1024)
                for u in range(2):
                    n = (g * NPAIR + mm) * 2 + u
                    for q in range(2):
                        nc.tensor.matmul(
                            pt[:, q * 512 + u * JS : q * 512 + (u + 1) * JS],
                            lhsT=x1T[
                                q * 64 : (q + 1) * 64, n * 128 : (n + 1) * 128
                            ],
                            rhs=w2dup[q * 64 : (q + 1) * 64, :],
                            start=True,
                            stop=True,
                        )
                ot_v = ot[:, mm * 1024 : (mm + 1) * 1024].rearrange(
                    "p (u q j) -> p q u j", u=2, q=2
                )
                xt_v = xt[:, mm * 1024 : (mm + 1) * 1024].rearrange(
                    "p (u q j) -> p q u j", u=2, q=2
                )
                if mm == NPAIR - 1:
                    # direct f32-PSUM multiply on DVE (no ACT copy); one
                    # pair per group keeps DVE and ACT loads balanced, and
                    # putting it last shortens the half-1 store's chain
                    nc.vector.tensor_mul(
                        ot_v,
                        pt[:].rearrange("p (q u j) -> p q u j", q=2, u=2),
                        xt_v,
                    )
                else:
                    yb = yb_pool.tile([128, 4 * JS], bf16, name="yb", tag="yb")
                    nc.scalar.copy(yb[:], pt[:])
                    nc.vector.tensor_mul(
                        ot_v,
                        yb[:].rearrange("p (q u j) -> p q u j", q=2, u=2),
                        xt_v,
                    )
                if mm % 2 == 1:
                    half = mm // 2
                    nc.sync.dma_start(
                        out=out[g][:, half * 2048 : (half + 1) * 2048],
                        in_=ot[:, half * 2048 : (half + 1) * 2048],
                    )
            if g + XBUFS < NG:
                load(g + XBUFS)

        for g in range(NG):
            process(g)

    nc.compile()
    return nc


def _get_nc():
    if "nc" not in _CACHE:
        _CACHE["nc"] = _build_nc()
    return _CACHE["nc"]


def _make_in_maps(input1, input2, weights):
    import ml_dtypes

    BF = ml_dtypes.bfloat16
    input1 = np.asarray(input1, dtype=np.float32)
    input2 = np.asarray(input2, dtype=np.float32)
    weights = np.asarray(weights, dtype=np.float32)

    # x1t[q*64+i, n*128+p] = input1[n*256 + 2p + q, i]
    x1t = (
        input1.reshape(NSUP, 128, 2, I)
        .transpose(2, 3, 0, 1)
        .reshape(128, NSUP * 128)
        .astype(BF)
    )

    in_maps = []
    for c in range(NCORES):
        sl = slice(c * JS, (c + 1) * JS)
        # wd[h*64+i, j''*64+k] = weights[i, c*JS + h*128 + j'', k]
        wd = (
            weights[:, sl, :]
            .reshape(I, 2, 128, K)
            .transpose(1, 0, 2, 3)
            .reshape(128, 128 * K)
            .astype(BF)
        )
        # x2d[g, p, (s*2+q)*256+j] = input2[(g*8+s)*256 + 2p + q, sl][j]
        x2d = (
            input2[:, sl]
            .reshape(NG, GROUP, 128, 2, JS)
            .transpose(0, 2, 1, 3, 4)
            .reshape(NG, 128, GFREE)
            .astype(BF)
        )
        in_maps.append({"input1": x1t, "input2": x2d, "weights": wd})
    return in_maps


def run(input1, input2, weights, trace=False, **spmd_kwargs):
    from concourse.bass_utils import run_bass_kernel_spmd

    nc = _get_nc()
    in_maps = _make_in_maps(input1, input2, weights)
    res = run_bass_kernel_spmd(
        nc, in_maps, core_ids=list(range(NCORES)), trace=trace, **spmd_kwargs
    )
    outs = []
    for c in range(NCORES):
        o = np.asarray(res.results[c]["out"])  # (NG, 128, GFREE) bf16
        outs.append(
            o.reshape(NG, 128, GROUP, 2, JS)
            .transpose(0, 2, 1, 3, 4)
            .reshape(B, JS)
        )
    full = np.concatenate(outs, axis=1).astype(np.float32)
    return full, res


def kernel(input1, input2, weights):
    full, _ = run(input1, input2, weights, trace=False)
    return full


# revision 21
# speedup vs baseline: 1.0184x; 1.0051x over previous
"""Trainium2 Bass kernel for nn_Bilinear_54065048322517.

Math:  out[b, j] = input2[b, j] * sum_{i,k} weights[i, j, k] * input1[b, i]
           =   input2 * (input1 @ weights.sum(axis=2))
Shapes: input1 (16384, 64) f32, input2 (16384, 2048) f32,
        weights (64, 2048, 64) f32, out (16384, 2048) f32.

Sharding: split J=2048 into 8 shards of 256 (one per NeuronCore);
J-sharding avoids replicating the 32MB weights tensor.

Precision: the HBM side runs in bf16, halving traffic to 20MB per
core: input1 2MB + input2 shard 8MB + weights shard 2MB read, out
shard 8MB written.  The K-reduction and GEMM accumulate in f32 (DVE
reduce -> f32, matmul -> PSUM f32); bf16 rounding enters only on the
stored operands and the final output (rel-l2 vs the f32 reference
~4e-3, well under the 2e-2 gate).  The host casts/gathers.

Host staging puts every per-core DRAM array in the exact layout the
kernel consumes, so all DMAs move 2-16KB contiguous runs/partition:
  x1t[q*64+i, n*128+p]       = input1[n*256 + 2p + q, i]
  wd[h*64+i, j''*64+k]       = weights[i, jsl(h*128+j''), k]
  x2d[g, p, (s*2+q)*256+j]   = input2[(g*8+s)*256 + 2p + q, jsl(j)]
  (out is produced in the x2d layout and un-permuted on the host)

Per-core kernel:
  - wd loaded in 8 chunks of 256KB alternating between the two HWDGE
    rings (scalar/ACT gets even chunks, sync/SP odd), each reduced
    over k by DVE as it lands -> w2h (128, 128) f32, partition h*64+i
    holds w2[i, h-half cols].  Pipelining the reduce under the loads
    keeps the w2 chain off the critical path (v1 lost ~12us here).
  - two selection-matrix matmuls (sel_h[h*64+i, q*64+i] = 1) fan
    w2h out to pdup[q*64+i, h*128+j''] = w2[i, j'] (PSUM f32); one
    DVE copy casts to w2dup (128, 256) bf16 so each PE row-group q
    holds the full reduced weight matrix.
  - x1t loads on the sync ring behind the odd w chunks (0.5MB head
    chunk covering groups 0-1 first, then the 1.5MB rest).
  - 8 groups of 8 super-tiles (256 rows each): per group one 1MB x2
    load (scalar ring); per super-tile PAIR four matmuls (q=0/q=1 on
    disjoint PE row-groups run concurrently; the two u-steps share a
    PSUM bank half each: col q*512 + u*256) into a 2-bank PSUM tile,
    then ONE DVE multiply for the pair (1024 elems/partition -- half
    the per-instruction read-write-bubble overhead of per-super-tile
    muls); two 512KB half-group stores (sync ring).
"""

import numpy as np

B, I, J, K = 16384, 64, 2048, 64
NCORES = 8
JS = J // NCORES          # 256 columns per core
NSUP = B // 256           # 64 super-tiles of 256 rows
GROUP = 8                 # super-tiles per DMA group
NG = NSUP // GROUP        # 8 groups
NPAIR = GROUP // 2        # super-tile pairs per group
XBUFS = 4                 # xtile buffer depth
OBUFS = 3                 # otile buffer depth
NWCHUNK = 8               # weights load chunks
GFREE = GROUP * 2 * JS    # 4096 elems per partition per group

_CACHE = {}


def _build_nc():
    from contextlib import ExitStack

    import concourse.mybir as mybir
    import concourse.tile as tile
    from concourse import bacc

    f32 = mybir.dt.float32
    bf16 = mybir.dt.bfloat16
    nc = bacc.Bacc()

    x1 = nc.dram_tensor("input1", [128, NSUP * 128], bf16, kind="ExternalInput")
    x2 = nc.dram_tensor("input2", [NG, 128, GFREE], bf16, kind="ExternalInput")
    w = nc.dram_tensor("weights", [128, (JS // 2) * K], bf16, kind="ExternalInput")
    out = nc.dram_tensor("out", [NG, 128, GFREE], bf16, kind="ExternalOutput")

    with tile.TileContext(nc) as tc, ExitStack() as ctx:
        const_pool = ctx.enter_context(tc.tile_pool(name="const", bufs=1))
        wc_pool = ctx.enter_context(tc.tile_pool(name="wc", bufs=1))
        x_pool = ctx.enter_context(tc.tile_pool(name="xin", bufs=XBUFS))
        o_pool = ctx.enter_context(tc.tile_pool(name="oout", bufs=OBUFS))
        yb_pool = ctx.enter_context(tc.tile_pool(name="yb", bufs=3))
        ps_pool = ctx.enter_context(tc.tile_pool(name="ps", bufs=3, space="PSUM"))
        tr_pool = ctx.enter_context(tc.tile_pool(name="tr", bufs=1, space="PSUM"))

        # selection masks: sel_h[h*64+i, q*64+i] = 1, else 0
        # (sel_h.T @ w2h)[q*64+i, j''] = w2h[h*64+i, j'']
        sel = []
        for h in range(2):
            sh = const_pool.tile([128, 128], f32, name=f"sel{h}")
            sel.append(sh)
            nc.gpsimd.memset(sh[:], 0.0)
            for q in range(2):
                # fill where p - 64h - m' == 0 over the (128, 64) column block
                nc.gpsimd.affine_select(
                    out=sh[:, q * 64 : (q + 1) * 64],
                    in_=sh[:, q * 64 : (q + 1) * 64],
                    compare_op=mybir.AluOpType.not_equal,
                    fill=1.0,
                    base=-64 * h,
                    pattern=[[-1, 64]],
                    channel_multiplier=1,
                )

        # ---- weights: 8 chunk loads front-loaded on both rings, then
        # pipelined DVE reduce (chunks 0-3 head the scalar ring, 4-7 the
        # sync ring, so w gets both rings' full bandwidth first) ----
        w2h = const_pool.tile([128, JS // 2], f32)  # (128, 128), part h*64+i
        wcsz = (JS // 2) * K // NWCHUNK  # 1024 elems per partition per chunk
        jcs = (JS // 2) // NWCHUNK       # 16 w2h columns per chunk
        wchunks = []
        for c in range(NWCHUNK):
            wchunk = wc_pool.tile([128, wcsz], bf16, name=f"wchunk{c}", tag=f"wc{c}")
            wchunks.append(wchunk)
            # interleave ring order with landing order: c even -> scalar,
            # c odd -> sync, so reduces can run in emission order c=0..7
            eng = nc.scalar if c % 2 == 0 else nc.sync
            eng.dma_start(out=wchunk[:], in_=w[:, c * wcsz : (c + 1) * wcsz])
        for c in range(NWCHUNK):
            nc.vector.tensor_reduce(
                out=w2h[:, c * jcs : (c + 1) * jcs],
                in_=wchunks[c][:].rearrange("p (j k) -> p j k", k=K),
                axis=mybir.AxisListType.X,
                op=mybir.AluOpType.add,
            )

        # ---- x1t load on sync ring (head chunk first for groups 0-1) ----
        x1T = const_pool.tile([128, NSUP * 128], bf16)
        nc.sync.dma_start(out=x1T[:, 0:2048], in_=x1[:, 0:2048])
        nc.sync.dma_start(out=x1T[:, 2048:], in_=x1[:, 2048:])

        # ---- x2 group prefetch (scalar ring) ----
        xtiles = []

        def load(g):
            assert len(xtiles) == g
            xt = x_pool.tile([128, GFREE], bf16, name=f"xt{g}", tag="xt")
            xtiles.append(xt)
            nc.scalar.dma_start(out=xt[:], in_=x2[g])

        for g in range(XBUFS):
            load(g)

        # ---- de-interleave + duplicate w2 via selection matmuls ----
        # pdup[q*64+i, h*128+j''] = w2h[h*64+i, j'']
        pdup = tr_pool.tile([128, JS], f32)
        for h in range(2):
            nc.tensor.matmul(
                pdup[:, h * 128 : (h + 1) * 128],
                lhsT=sel[h][:],
                rhs=w2h[:],
                start=True,
                stop=True,
            )
        w2dup = const_pool.tile([128, JS], bf16)
        nc.vector.tensor_copy(out=w2dup[:], in_=pdup[:])

        # ---- main loop ----
        # Per super-tile pair: 4 matmuls -> PSUM f32; ScalarE (ACT, idle
        # otherwise, has the PSUM port) casts the pair to SBUF bf16; DVE
        # multiplies bf16*bf16 -> bf16 at its 2x 16-bit rate.
        def process(g):
            xt = xtiles[g]
            ot = o_pool.tile([128, GFREE], bf16, name=f"ot{g}", tag="ot")
            for mm in range(NPAIR):
                # 4 matmuls into a 2-bank PSUM tile: col q*512 + u*256;
                # the q pair runs concurrently on disjoint PE row-groups
                # into different banks, u-steps fill the bank halves.
                pt = ps_pool.tile([128, 4 * JS], f32)  # (128, 1024)
                for u in range(2):
                    n = (g * NPAIR + mm) * 2 + u
                    for q in range(2):
                        nc.tensor.matmul(
                            pt[:, q * 512 + u * JS : q * 512 + (u + 1) * JS],
                            lhsT=x1T[
                                q * 64 : (q + 1) * 64, n * 128 : (n + 1) * 128
                            ],
                            rhs=w2dup[q * 64 : (q + 1) * 64, :],
                            start=True,
                            stop=True,
                        )
                ot_v = ot[:, mm * 1024 : (mm + 1) * 1024].rearrange(
                    "p (u q j) -> p q u j", u=2, q=2
                )
                xt_v = xt[:, mm * 1024 : (mm + 1) * 1024].rearrange(
                    "p (u q j) -> p q u j", u=2, q=2
                )
                yb = yb_pool.tile([128, 4 * JS], bf16, name="yb", tag="yb")
                nc.scalar.copy(yb[:], pt[:])
                nc.vector.tensor_mul(
                    ot_v,
                    yb[:].rearrange("p (q u j) -> p q u j", q=2, u=2),
                    xt_v,
                )
                if mm % 2 == 1:
                    half = mm // 2
                    nc.sync.dma_start(
                        out=out[g][:, half * 2048 : (half + 1) * 2048],
                        in_=ot[:, half * 2048 : (half + 1) * 2048],
                    )
            if g + XBUFS < NG:
                load(g + XBUFS)

        for g in range(NG):
            process(g)

    nc.compile()
    return nc


def _get_nc():
    if "nc" not in _CACHE:
        _CACHE["nc"] = _build_nc()
    return _CACHE["nc"]


def _make_in_maps(input1, input2, weights):
    import ml_dtypes

    BF = ml_dtypes.bfloat16
    input1 = np.asarray(input1, dtype=np.float32)
    input2 = np.asarray(input2, dtype=np.float32)
    weights = np.asarray(weights, dtype=np.float32)

    # x1t[q*64+i, n*128+p] = input1[n*256 + 2p + q, i]
    x1t = (
        input1.reshape(NSUP, 128, 2, I)
        .transpose(2, 3, 0, 1)
        .reshape(128, NSUP * 128)
        .astype(BF)
    )

    in_maps = []
    for c in range(NCORES):
        sl = slice(c * JS, (c + 1) * JS)
        # wd[h*64+i, j''*64+k] = weights[i, c*JS + h*128 + j'', k]
        wd = (
            weights[:, sl, :]
            .reshape(I, 2, 128, K)
            .transpose(1, 0, 2, 3)
            .reshape(128, 128 * K)
            .astype(BF)
        )
        # x2d[g, p, (s*2+q)*256+j] = input2[(g*8+s)*256 + 2p + q, sl][j]
        x2d = (
            input2[:, sl]
            .reshape(NG, GROUP, 128, 2, JS)
            .transpose(0, 2, 1, 3, 4)
            .reshape(NG, 128, GFREE)
            .astype(BF)
        )
        in_maps.append({"input1": x1t, "input2": x2d, "weights": wd})
    return in_maps


def run(input1, input2, weights, trace=False, **spmd_kwargs):
    from concourse.bass_utils import run_bass_kernel_spmd

    nc = _get_nc()
    in_maps = _make_in_maps(input1, input2, weights)
    res = run_bass_kernel_spmd(
        nc, in_maps, core_ids=list(range(NCORES)), trace=trace, **spmd_kwargs
    )
    outs = []
    for c in range(NCORES):
        o = np.asarray(res.results[c]["out"])  # (NG, 128, GFREE) bf16
        outs.append(
            o.reshape(NG, 128, GROUP, 2, JS)
            .transpose(0, 2, 1, 3, 4)
            .reshape(B, JS)
        )
    full = np.concatenate(outs, axis=1).astype(np.float32)
    return full, res


def kernel(input1, input2, weights):
    full, _ = run(input1, input2, weights, trace=False)
    return full
